# revision 7
# baseline (speedup 1.0000x reference)
"""Converged Toeplitz inhibition kernel for TRN2 (8 NeuronCores, SPMD).

out[n, c, h, w] = sum_k act[n, k, h, w] * Winv[k, c]
where Winv = inv(I - circulant(pad_roll(inhibition_filter, C)))  [C x C]

Strategy (per sharding hint): the tiny C x C inverse is computed on the host
and replicated to every core; activations are sharded along batch N (32 -> 4
per core). Each core runs a dense [K=256] x [M=256] x [N_free=4*4096] matmul.

The kernel is HBM-bandwidth-bound (~358 GB/s per NeuronCore), so all wire
traffic is fp16: activations are cast to fp16 on the host, the matmul runs
fp16 x fp16 -> fp32 PSUM, and the output is written back as fp16 and upcast
to fp32 on the host. This halves HBM traffic vs fp32 (16.8 MB/core total)
for a ~47 us DMA floor; fp16's 11-bit mantissa keeps rel err ~1e-3, far
under the 2e-2 gate (the old fp32r path also had an 11-bit mantissa).

  - weights held in SBUF as four 128x128 views of one [128, 512] tile
  - activations DMA'd in as [128, 2048] fp16 chunks (0.5 MB) on the SP
    HWDGE ring
  - PE matmul fp16, free dim 512, PSUM fp32
  - PSUM -> SBUF evacuation (with fp32->fp16 cast) alternating
    ScalarE / VectorE
  - output DMA'd out as [128, 2048] fp16 chunks on the ACT HWDGE ring, so
    read and write streams overlap
"""

import numpy as np

import concourse.bass as bass
import concourse.bacc as bacc
import concourse.mybir as mybir
import concourse.tile as tile
from concourse.bass_utils import run_bass_kernel_spmd

N, C, H, W = 32, 256, 64, 64
HW = H * W  # 4096
NCORES = 8
NB = N // NCORES  # batches per core
P = 128  # partitions
FD = 512  # matmul free dim (one fp32 PSUM bank)

MM_DT = mybir.dt.float16


def _build_w(inhibition_filter: np.ndarray) -> np.ndarray:
    """Replicates reference._pad_roll + _circulant + inv(I - tpl) in numpy."""
    filt = np.asarray(inhibition_filter, dtype=np.float32)
    scope = filt.shape[0]
    pad_left = (C - scope) // 2
    padded = np.zeros(C, np.float32)
    padded[pad_left : pad_left + scope] = filt
    kernel = np.roll(padded, C // 2 + 1)
    idx = (np.arange(C)[None, :] - np.arange(C)[:, None]) % C
    tpl = kernel[idx].astype(np.float64)
    w = np.linalg.inv(np.eye(C, dtype=np.float64) - tpl)
    return np.ascontiguousarray(w.astype(np.float32))


def _body(tc: tile.TileContext, out, act, w):
    # In-DMAs ride the SP HWDGE ring (nc.sync), out-DMAs the ACT ring
    # (nc.scalar) so input and output streams don't serialize on one FIFO
    # ring.
    nc = tc.nc
    CH = 2048  # chunk width, 4 matmul slices per chunk
    NCH = HW // CH  # 2 chunks
    JPC = CH // FD  # 4 matmul free-dim slices per chunk
    with (
        tc.tile_pool(name="wpool", bufs=1) as wpool,
        tc.tile_pool(name="apool", bufs=3) as apool,
        tc.tile_pool(name="opool", bufs=3) as opool,
        tc.tile_pool(name="psum", bufs=2, space="PSUM") as pspool,
    ):
        # Weights arrive host-packed as [128, 1024]: the four 128x128 tiles
        # (k-major, then m) side by side, so one DMA with 2 KB lines loads
        # them all. It rides the sync ring FIRST so weights are resident
        # before the first activation chunk lands and matmuls start
        # immediately.
        wtile = wpool.tile([P, 4 * P], MM_DT, tag="w", name="wtile")
        nc.sync.dma_start(out=wtile[:], in_=w[:, :])
        wt = [
            [wtile[:, (2 * k + m) * P : (2 * k + m + 1) * P] for m in range(2)]
            for k in range(2)
        ]

        for n in range(NB):
            a = {}
            for c in range(NCH):
                for k in range(2):
                    a[k, c] = apool.tile([P, CH], MM_DT, tag=f"a{k}{c}", name=f"a{k}{c}")
                    nc.sync.dma_start(
                        out=a[k, c][:],
                        in_=act[n, k * P : (k + 1) * P, c * CH : (c + 1) * CH],
                    )
            for c in range(NCH):
                for m in range(2):
                    o = opool.tile([P, CH], MM_DT, tag=f"o{m}{c}", name=f"o{m}{c}", bufs=3 if c == 0 else 2)
                    ps = [
                        pspool.tile([P, FD], mybir.dt.float32, name=f"ps{jj}")
                        for jj in range(JPC)
                    ]
                    # k-outer: 4 consecutive matmuls share one LDWEIGHTS,
                    # accumulating across k into 4 PSUM banks.
                    for k in range(2):
                        for jj in range(JPC):
                            nc.tensor.matmul(
                                ps[jj][:],
                                lhsT=wt[k][m],
                                rhs=a[k, c][:, jj * FD : (jj + 1) * FD],
                                start=(k == 0),
                                stop=(k == 1),
                            )
                    for jj in range(JPC):
                        if jj % 2 == 0:
                            nc.scalar.copy(o[:, jj * FD : (jj + 1) * FD], ps[jj][:])
                        else:
                            nc.vector.tensor_copy(o[:, jj * FD : (jj + 1) * FD], ps[jj][:])
                    nc.scalar.dma_start(
                        out=out[n, m * P : (m + 1) * P, c * CH : (c + 1) * CH],
                        in_=o[:],
                    )


_NC_CACHE = None


def _get_nc():
    global _NC_CACHE
    if _NC_CACHE is None:
        nc = bacc.Bacc(
            "TRN2", debug=False, enable_asserts=False, enable_partition_id=False
        )
        act = nc.dram_tensor("act", [NB, C, HW], MM_DT, kind="ExternalInput").ap()
        w = nc.dram_tensor("w", [P, 4 * P], MM_DT, kind="ExternalInput").ap()
        out = nc.dram_tensor("out", [NB, C, HW], MM_DT, kind="ExternalOutput").ap()
        with tile.TileContext(nc) as tc:
            _body(tc, out, act, w)
        nc.compile()
        _NC_CACHE = nc
    return _NC_CACHE


def _run(activations: np.ndarray, w: np.ndarray, trace: bool = False):
    acts = (
        np.ascontiguousarray(activations, dtype=np.float32)
        .astype(np.float16)
        .reshape(NCORES, NB, C, HW)
    )
    # Pack w [256, 256] into [128, 1024]: four 128x128 tiles (k-major, then
    # m) side by side, matching the single weight DMA + wt views on-device.
    w16 = w.astype(np.float16)
    wp = np.empty((P, 4 * P), np.float16)
    for k in range(2):
        for m in range(2):
            wp[:, (2 * k + m) * P : (2 * k + m + 1) * P] = w16[
                k * P : (k + 1) * P, m * P : (m + 1) * P
            ]
    wp = np.ascontiguousarray(wp)
    in_maps = [{"act": acts[i], "w": wp} for i in range(NCORES)]
    nc = _get_nc()
    res = run_bass_kernel_spmd(nc, in_maps, list(range(NCORES)), trace=trace)
    out = np.concatenate([res.results[i]["out"] for i in range(NCORES)], axis=0)
    return out.astype(np.float32).reshape(N, C, H, W), res


def kernel(activations: np.ndarray, inhibition_filter: np.ndarray) -> np.ndarray:
    w = _build_w(inhibition_filter)
    out, _ = _run(activations, w, trace=False)
    return out
